# revision 25
# baseline (speedup 1.0000x reference)
"""Trainium2 Bass kernel: additive (Bahdanau-style) attention layer.

reference:
    wf    = features @ Wk + Wb            # [B, T, U]
    uh    = hidden @ Uk + Ub              # [B, 1, U]
    score = tanh(wf + uh)                 # [B, T, U]
    logit = score @ Vk + Vb               # [B, T, 1]
    attn  = softmax(logit, axis=T)
    ctx   = sum_T(attn * features)        # [B, D]
    returns (ctx, attn)

Strategy: pure data-parallel over batch (32 -> 4 per core, 8 cores, no
collectives).  Per core, everything is computed in "layout B" where the
unit axis U lives on SBUF partitions:

    fT[d, t]  = transpose(features)      via TensorE transposes (bf16)
    wfT[u, t] = sum_d Wk[d,u] * fT[d,t]  TensorE, PSUM accumulate
    scoreT    = tanh(wfT + bias[u])      ScalarE, bias = (uh+Ub+Wb)^T per
                                         partition, fused PSUM->SBUF
    logit[t]  = sum_u Vk[u]*scoreT[u,t]  TensorE (M=1)
    softmax over T in [4, 2048] layout   VectorE/ScalarE
    ctx[d]    = sum_t attn[t]*f[t,d]     TensorE (lhsT = attn column)

Vb is mathematically irrelevant (softmax shift invariance) and ignored.
Compute dtype bf16 (cast during DMA), accumulation fp32.
"""

import numpy as np

B, T, D, H, U = 32, 2048, 512, 512, 512
N_CORES = 8
BPC = B // N_CORES      # batches per core
P = 128                 # partitions
NG = 4                  # token groups per batch
GT = T // NG            # tokens per group (512)
NT = GT // P            # token tiles per group (4)
DB = D // P             # d blocks (4)
UB = U // P             # u blocks (4)
HB = H // P             # h blocks (4)
TPB = T // P            # token tiles per batch (16)

_CACHE = {}


def _build():
    import concourse.bacc as bacc
    import concourse.tile as tile
    from concourse import mybir
    from concourse.masks import make_identity

    f32 = mybir.dt.float32
    bf16 = mybir.dt.bfloat16
    AF = mybir.ActivationFunctionType

    nc = bacc.Bacc(
        "TRN2",
        target_bir_lowering=False,
        debug=False,
        num_devices=N_CORES,
    )

    feat = nc.declare_dram_parameter("features", [BPC * T, D], f32, isOutput=False).ap()
    hid = nc.declare_dram_parameter("hidden", [BPC, H], f32, isOutput=False).ap()
    wk_d = nc.declare_dram_parameter("Wk", [D, U], f32, isOutput=False).ap()
    wb_d = nc.declare_dram_parameter("Wb", [U], f32, isOutput=False).ap()
    uk_d = nc.declare_dram_parameter("Uk", [H, U], f32, isOutput=False).ap()
    ub_d = nc.declare_dram_parameter("Ub", [U], f32, isOutput=False).ap()
    vk_d = nc.declare_dram_parameter("Vk", [U, 1], f32, isOutput=False).ap()
    # outputs are pre-normalization: host divides by s (softmax denominator)
    ctx_out = nc.declare_dram_parameter("ctx", [BPC, D], f32, isOutput=True).ap()
    attn_out = nc.declare_dram_parameter("attn", [BPC, T], f32, isOutput=True).ap()
    s_out = nc.declare_dram_parameter("s", [BPC, 1], f32, isOutput=True).ap()

    with tile.TileContext(nc) as tc:
        with (
            tc.tile_pool(name="const", bufs=1) as cpool,
            tc.tile_pool(name="ftp", bufs=1) as ftpool,
            tc.tile_pool(name="fTp", bufs=2) as fTpool,
            tc.tile_pool(name="scp", bufs=2) as scpool,
            tc.tile_pool(name="ps_fT", bufs=2, space="PSUM") as ps_fT,
            tc.tile_pool(name="ps_wfT", bufs=2, space="PSUM") as ps_wfT,
            tc.tile_pool(name="ps_mm1", bufs=2, space="PSUM") as ps_mm1,
            tc.tile_pool(name="ps_cx", bufs=1, space="PSUM") as ps_cx,
            tc.tile_pool(name="ps_pre", bufs=1, space="PSUM") as ps_pre,
        ):
            # ---------- constants ----------
            # identities first: tiny gpsimd work, unblocks all PE transposes
            ident32 = cpool.tile([P, P], f32)
            make_identity(nc, ident32[:])
            ident16 = cpool.tile([P, P], bf16)
            make_identity(nc, ident16[:])

            # ---------- features: load all (resident), f32 -> bf16 in DMA ----
            # right after the identities so the gpsimd SWDGE queue works on
            # them immediately (descriptor gen is the serial cost).
            # Layout: within a 512-token group, partition p slot i holds token
            # 4p+i, so each partition reads ONE contiguous 8KB chunk from HBM
            # (fast descriptors). Tokens come out permuted; the host fixes the
            # attention-weight order, and context is order-invariant.
            ft = {}
            for b in range(BPC):
                for g in range(NG):
                    t = ftpool.tile([P, NT, D], bf16, tag=f"ft_{b}_{g}")
                    src = feat[b * T + g * GT: b * T + (g + 1) * GT, :]
                    nc.gpsimd.dma_start(t[:], src.rearrange("(p i) d -> p i d", i=NT))
                    ft[(b, g)] = t

            # small sync DMAs first (hidden unblocks the preamble PE work),
            # then weights; wk/vk cast to bf16 on DVE
            hid_sb = cpool.tile([BPC, H], f32)
            nc.sync.dma_start(hid_sb[:], hid[:, :])
            vkT32 = cpool.tile([P, UB], f32)
            nc.sync.dma_start(vkT32[:], vk_d.rearrange("(b p) o -> p (b o)", p=P))
            vkT = cpool.tile([P, UB], bf16)
            nc.vector.tensor_copy(vkT[:], vkT32[:])
            wbT = cpool.tile([P, UB], f32)
            nc.sync.dma_start(wbT[:], wb_d.rearrange("(b p) -> p b", p=P))
            ubT = cpool.tile([P, UB], f32)
            nc.sync.dma_start(ubT[:], ub_d.rearrange("(b p) -> p b", p=P))
            wubT = cpool.tile([P, UB], f32)
            nc.vector.tensor_add(wubT[:], wbT[:], ubT[:])
            wk_sb = []
            for j in range(DB):
                tf = cpool.tile([P, U], f32, tag=f"wk32_{j}")
                nc.sync.dma_start(tf[:], wk_d[j * P:(j + 1) * P, :])
                t = cpool.tile([P, U], bf16, tag=f"wk{j}")
                nc.vector.tensor_copy(t[:], tf[:])
                wk_sb.append(t)
            uk_sb = []
            for j in range(HB):
                t = cpool.tile([P, U], f32, tag=f"uk{j}")
                nc.sync.dma_start(t[:], uk_d[j * P:(j + 1) * P, :])
                uk_sb.append(t)

            # ---------- preamble A: hidden transpose (only needs hid_sb) ----
            hT_ps = ps_pre.tile([P, HB * BPC], f32, tag="pre")
            for j in range(HB):
                nc.tensor.transpose(
                    hT_ps[:, j * BPC:(j + 1) * BPC],
                    hid_sb[:, j * P:(j + 1) * P],
                    ident32[:BPC, :BPC],
                )
            hT_sb = cpool.tile([P, HB * BPC], f32)
            nc.vector.tensor_copy(hT_sb[:], hT_ps[:])

            # batch b's exp(logits) live at partition 32*b (legal base partitions).
            # softmax is shift-invariant and |logit| <~ 2 here, so exp is taken
            # directly (no max subtraction), fused into the PSUM->SBUF copy.
            e_sb = cpool.tile([P, T], f32)
            s_part = cpool.tile([P, NG], f32)     # per-group partial sums
            s_sb = cpool.tile([P, 1], f32)
            aT_sb = cpool.tile([P, BPC * TPB], bf16)   # col = b*TPB + i
            ctx_sb = cpool.tile([P, D], f32)

            # ---------- main pipeline over 16 groups, 2-deep skew ----------
            groups = [(b, g) for b in range(BPC) for g in range(NG)]
            fT_of = {}
            sc_of = {}

            def stage_T(idx):
                b, g = groups[idx]
                fgrp = ft[(b, g)]
                tiles = []
                for j in range(DB):
                    fT_ps = ps_fT.tile([P, GT], bf16, tag="fTps")
                    for i in range(NT):
                        nc.tensor.transpose(
                            fT_ps[:, i * P:(i + 1) * P],
                            fgrp[:, i, j * P:(j + 1) * P],
                            ident16[:, :],
                        )
                    t = fTpool.tile([P, GT], bf16, tag=f"fT{j}")
                    nc.vector.tensor_copy(t[:], fT_ps[:])
                    tiles.append(t)
                fT_of[idx] = tiles

            def stage_MM(idx):
                b, g = groups[idx]
                tiles = fT_of.pop(idx)
                scs = []
                for ub_i in range(UB):
                    wfT_ps = ps_wfT.tile([P, GT], f32, tag="wfT")
                    for j in range(DB):
                        nc.tensor.matmul(
                            wfT_ps[:],
                            wk_sb[j][:, ub_i * P:(ub_i + 1) * P],
                            tiles[j][:],
                            start=(j == 0),
                            stop=(j == DB - 1),
                        )
                    sc = scpool.tile([P, GT], bf16, tag=f"sc{ub_i}")
                    nc.scalar.activation(
                        sc[:],
                        wfT_ps[:],
                        AF.Tanh,
                        bias=bias_sb[:, ub_i * BPC + b: ub_i * BPC + b + 1],
                        scale=1.0,
                    )
                    scs.append(sc)
                sc_of[idx] = scs

            def stage_LG(idx):
                b, g = groups[idx]
                scs = sc_of.pop(idx)
                lg_ps = ps_mm1.tile([1, GT], f32, tag="mm1")
                for ub_i in range(UB):
                    nc.tensor.matmul(
                        lg_ps[:],
                        vkT[:, ub_i:ub_i + 1],
                        scs[ub_i][:],
                        start=(ub_i == 0),
                        stop=(ub_i == UB - 1),
                    )
                # fused exp straight from PSUM; accum_out = per-group sum
                nc.scalar.activation(
                    e_sb[32 * b:32 * b + 1, g * GT:(g + 1) * GT], lg_ps[:],
                    AF.Exp, bias=0.0, scale=1.0,
                    accum_out=s_part[32 * b:32 * b + 1, g:g + 1],
                )

            cx_of = {}

            def stage_CTX(idx):
                # incremental attn-transpose + context accumulation for one
                # group (runs one group behind LG so exp() is long done)
                b, g = groups[idx]
                r32 = slice(32 * b, 32 * b + 1)
                aT_ps = ps_pre.tile([P, NT], f32, tag="pre")
                for i in range(NT):
                    nc.tensor.transpose(
                        aT_ps[:, i:i + 1],
                        e_sb[r32, g * GT + i * P: g * GT + (i + 1) * P],
                        ident32[r32, 32 * b:32 * b + 1],
                        tile_position=(32 * b, 0),
                    )
                cols = slice(b * TPB + g * NT, b * TPB + (g + 1) * NT)
                nc.vector.tensor_copy(aT_sb[:, cols], aT_ps[:])
                if g == 0:
                    cx_of[b] = ps_cx.tile([1, D], f32, tag="cx", name=f"cx_ps{b}")
                cx_ps = cx_of[b]
                for i in range(NT):
                    nc.tensor.matmul(
                        cx_ps[:],
                        aT_sb[:, b * TPB + g * NT + i: b * TPB + g * NT + i + 1],
                        ft[(b, g)][:, i, :],
                        start=(g == 0 and i == 0),
                        stop=(g == NG - 1 and i == NT - 1),
                    )
                if g == NG - 1:
                    nc.vector.reduce_sum(
                        s_sb[r32, :], s_part[r32, :], axis=mybir.AxisListType.X
                    )
                    nc.sync.dma_start(attn_out[b:b + 1, :], e_sb[r32, :])
                    nc.scalar.copy(ctx_sb[32 * b:32 * b + 1, :], cx_ps[:])
                    cx_of.pop(b)

            n = len(groups)
            # prologue: first two transpose stages, then the uh/bias preamble
            # (PE order: hT, T(0), T(1), uh-MM, uhT, MM(0) -- no long stalls)
            stage_T(0)
            stage_T(1)
            _PROLOGUE = True

            uh_ps = ps_pre.tile([BPC, U], f32, tag="pre")
            for j in range(HB):
                nc.tensor.matmul(
                    uh_ps[:],
                    hT_sb[:, j * BPC:(j + 1) * BPC],
                    uk_sb[j][:],
                    start=(j == 0),
                    stop=(j == HB - 1),
                )
            uh_sb = cpool.tile([BPC, U], f32)
            nc.vector.tensor_copy(uh_sb[:], uh_ps[:])
            uhT_ps = ps_pre.tile([P, UB * BPC], f32, tag="pre")
            for jb in range(UB):
                nc.tensor.transpose(
                    uhT_ps[:, jb * BPC:(jb + 1) * BPC],
                    uh_sb[:, jb * P:(jb + 1) * P],
                    ident32[:BPC, :BPC],
                )
            bias_sb = cpool.tile([P, UB * BPC], f32)
            for jb in range(UB):
                nc.vector.tensor_scalar_add(
                    bias_sb[:, jb * BPC:(jb + 1) * BPC],
                    uhT_ps[:, jb * BPC:(jb + 1) * BPC],
                    wubT[:, jb:jb + 1],
                )

            stage_MM(0)
            for k in range(2, n + 3):
                if k < n:
                    stage_T(k)
                if 1 <= k - 1 < n:
                    stage_MM(k - 1)
                if 0 <= k - 2 < n:
                    stage_LG(k - 2)
                if 0 <= k - 3 < n:
                    stage_CTX(k - 3)

            nc.sync.dma_start(ctx_out[:, :], ctx_sb[0:P:32, :])
            nc.sync.dma_start(s_out[:, :], s_sb[0:P:32, :])

    nc.compile()
    return nc


def _get_nc():
    if "nc" not in _CACHE:
        _CACHE["nc"] = _build()
    return _CACHE["nc"]


def _shard(features, hidden, Wk, Wb, Uk, Ub, Vk):
    f32 = np.float32
    features = np.asarray(features, dtype=f32).reshape(B, T, D)
    hidden = np.asarray(hidden, dtype=f32)
    Wk = np.ascontiguousarray(np.asarray(Wk, dtype=f32))
    Wb = np.ascontiguousarray(np.asarray(Wb, dtype=f32))
    Uk = np.ascontiguousarray(np.asarray(Uk, dtype=f32))
    Ub = np.ascontiguousarray(np.asarray(Ub, dtype=f32))
    Vk = np.ascontiguousarray(np.asarray(Vk, dtype=f32).reshape(U, 1))
    in_maps = []
    for c in range(N_CORES):
        sl = slice(c * BPC, (c + 1) * BPC)
        in_maps.append({
            "features": np.ascontiguousarray(features[sl]).reshape(BPC * T, D),
            "hidden": np.ascontiguousarray(hidden[sl]),
            "Wk": Wk, "Wb": Wb, "Uk": Uk, "Ub": Ub, "Vk": Vk,
        })
    return in_maps


def _run(in_maps, trace=False, tmpdir=None):
    from concourse.bass_utils import run_bass_kernel_spmd
    nc = _get_nc()
    return run_bass_kernel_spmd(
        nc, in_maps, core_ids=list(range(N_CORES)), trace=trace, tmpdir=tmpdir
    )


def _post(e, s, ctx_e):
    """Host-side gather math: un-permute tokens (device stores token 4p+i at
    group position i*128+p) and apply the softmax denominator."""
    nb = e.shape[0]
    attn = np.transpose(e.reshape(nb, NG, NT, P), (0, 1, 3, 2)).reshape(nb, T)
    attn = attn / s
    ctx = ctx_e / s
    return ctx, attn


def kernel(features, hidden, Wk, Wb, Uk, Ub, Vk, Vb=None, **_ignored):
    in_maps = _shard(features, hidden, Wk, Wb, Uk, Ub, Vk)
    res = _run(in_maps)
    ctx_e = np.concatenate([r["ctx"] for r in res.results], axis=0)
    e = np.concatenate([r["attn"] for r in res.results], axis=0)
    s = np.concatenate([r["s"] for r in res.results], axis=0)
    ctx, attn = _post(e, s, ctx_e)
    return ctx.astype(np.float32), attn.reshape(B, T, 1).astype(np.float32)


# revision 30
# speedup vs baseline: 1.4304x; 1.4304x over previous
"""Trainium2 Bass kernel: additive (Bahdanau-style) attention layer.

reference:
    wf    = features @ Wk + Wb            # [B, T, U]
    uh    = hidden @ Uk + Ub              # [B, 1, U]
    score = tanh(wf + uh)                 # [B, T, U]
    logit = score @ Vk + Vb               # [B, T, 1]
    attn  = softmax(logit, axis=T)
    ctx   = sum_T(attn * features)        # [B, D]
    returns (ctx, attn)

Strategy: pure data-parallel over batch (32 -> 4 per core, 8 cores, no
collectives).  Per core, everything is computed in "layout B" where the
unit axis U lives on SBUF partitions:

    fT[d, t]  = transpose(features)      via TensorE transposes (bf16)
    wfT[u, t] = sum_d Wk[d,u] * fT[d,t]  TensorE, PSUM accumulate
    scoreT    = tanh(wfT + bias[u])      ScalarE, bias = (uh+Ub+Wb)^T per
                                         partition, fused PSUM->SBUF
    logit[t]  = sum_u Vk[u]*scoreT[u,t]  TensorE (M=1)
    softmax over T in [4, 2048] layout   VectorE/ScalarE
    ctx[d]    = sum_t attn[t]*f[t,d]     TensorE (lhsT = attn column)

Vb is mathematically irrelevant (softmax shift invariance) and ignored.
Compute dtype bf16 (cast during DMA), accumulation fp32.
"""

import numpy as np

B, T, D, H, U = 32, 2048, 512, 512, 512
N_CORES = 8
BPC = B // N_CORES      # batches per core
P = 128                 # partitions
NG = 4                  # token groups per batch
GT = T // NG            # tokens per group (512)
NT = GT // P            # token tiles per group (4)
DB = D // P             # d blocks (4)
UB = U // P             # u blocks (4)
HB = H // P             # h blocks (4)
TPB = T // P            # token tiles per batch (16)

_CACHE = {}


def _build():
    import concourse.bacc as bacc
    import concourse.tile as tile
    from concourse import mybir
    from concourse.masks import make_identity

    f32 = mybir.dt.float32
    bf16 = mybir.dt.bfloat16
    AF = mybir.ActivationFunctionType

    nc = bacc.Bacc(
        "TRN2",
        target_bir_lowering=False,
        debug=False,
        num_devices=N_CORES,
    )

    feat = nc.declare_dram_parameter("features", [BPC * T, D], f32, isOutput=False).ap()
    hid = nc.declare_dram_parameter("hidden", [BPC, H], f32, isOutput=False).ap()
    wk_d = nc.declare_dram_parameter("Wk", [D, U], f32, isOutput=False).ap()
    wb_d = nc.declare_dram_parameter("Wb", [U], f32, isOutput=False).ap()
    uk_d = nc.declare_dram_parameter("Uk", [H, U], f32, isOutput=False).ap()
    ub_d = nc.declare_dram_parameter("Ub", [U], f32, isOutput=False).ap()
    vk_d = nc.declare_dram_parameter("Vk", [U, 1], f32, isOutput=False).ap()
    # outputs are pre-normalization: host divides by s (softmax denominator)
    ctx_out = nc.declare_dram_parameter("ctx", [BPC, D], f32, isOutput=True).ap()
    attn_out = nc.declare_dram_parameter("attn", [BPC, T], f32, isOutput=True).ap()
    s_out = nc.declare_dram_parameter("s", [BPC, 1], f32, isOutput=True).ap()

    with tile.TileContext(nc) as tc:
        with (
            tc.tile_pool(name="const", bufs=1) as cpool,
            tc.tile_pool(name="ftp", bufs=1) as ftpool,
            tc.tile_pool(name="fTp", bufs=2) as fTpool,
            tc.tile_pool(name="scp", bufs=2) as scpool,
            tc.tile_pool(name="ps_fT", bufs=2, space="PSUM") as ps_fT,
            tc.tile_pool(name="ps_wfT", bufs=2, space="PSUM") as ps_wfT,
            tc.tile_pool(name="ps_mm1", bufs=3, space="PSUM") as ps_mm1,
            tc.tile_pool(name="ps_pre", bufs=1, space="PSUM") as ps_pre,
        ):
            # ---------- constants ----------
            # identities first: tiny gpsimd work, unblocks all PE transposes
            ident32 = cpool.tile([P, P], f32)
            make_identity(nc, ident32[:])
            ident16 = cpool.tile([P, P], bf16)
            make_identity(nc, ident16[:])

            # ---------- features: load all (resident), f32 -> bf16 in DMA ----
            # right after the identities so the gpsimd SWDGE queue works on
            # them immediately (descriptor gen is the serial cost). The first
            # group is split into per-tile DMAs so transposes start sooner.
            ft = {}
            t00 = ftpool.tile([P, NT, D], bf16, tag="ft_0_0")
            for i in range(NT):
                src = feat[i * P: (i + 1) * P, :]
                nc.gpsimd.dma_start(t00[:, i, :], src)
            ft[(0, 0)] = t00
            for b in range(BPC):
                for g in range(NG):
                    if (b, g) == (0, 0):
                        continue
                    t = ftpool.tile([P, NT, D], bf16, tag=f"ft_{b}_{g}")
                    src = feat[b * T + g * GT: b * T + (g + 1) * GT, :]
                    nc.gpsimd.dma_start(t[:], src.rearrange("(i p) d -> p i d", p=P))
                    ft[(b, g)] = t

            # HAM warm-up: ~25 junk matmuls on the identity tile keep the PE
            # activity monitor busy while the first feature DMAs land, so real
            # work starts at full clock instead of 1.2 GHz.
            warm_ps = ps_wfT.tile([P, GT], f32, tag="wfT", name="warm_ps")
            for w in range(25):
                nc.tensor.matmul(
                    warm_ps[:, :P], ident32[:], ident32[:],
                    start=True, stop=True,
                )

            # small sync DMAs first (hidden unblocks the preamble PE work),
            # then weights; wk/vk cast to bf16 on DVE
            hid_sb = cpool.tile([BPC, H], f32)
            nc.sync.dma_start(hid_sb[:], hid[:, :])
            vkT32 = cpool.tile([P, UB], f32)
            nc.sync.dma_start(vkT32[:], vk_d.rearrange("(b p) o -> p (b o)", p=P))
            vkT = cpool.tile([P, UB], bf16)
            nc.vector.tensor_copy(vkT[:], vkT32[:])
            wbT = cpool.tile([P, UB], f32)
            nc.sync.dma_start(wbT[:], wb_d.rearrange("(b p) -> p b", p=P))
            ubT = cpool.tile([P, UB], f32)
            nc.sync.dma_start(ubT[:], ub_d.rearrange("(b p) -> p b", p=P))
            wubT = cpool.tile([P, UB], f32)
            nc.vector.tensor_add(wubT[:], wbT[:], ubT[:])
            wk_sb = []
            for j in range(DB):
                tf = cpool.tile([P, U], f32, tag=f"wk32_{j}")
                nc.sync.dma_start(tf[:], wk_d[j * P:(j + 1) * P, :])
                t = cpool.tile([P, U], bf16, tag=f"wk{j}")
                nc.vector.tensor_copy(t[:], tf[:])
                wk_sb.append(t)
            uk_sb = []
            for j in range(HB):
                t = cpool.tile([P, U], f32, tag=f"uk{j}")
                nc.sync.dma_start(t[:], uk_d[j * P:(j + 1) * P, :])
                uk_sb.append(t)

            # ---------- preamble A: hidden transpose (only needs hid_sb) ----
            hT_ps = ps_pre.tile([P, HB * BPC], f32, tag="pre")
            for j in range(HB):
                nc.tensor.transpose(
                    hT_ps[:, j * BPC:(j + 1) * BPC],
                    hid_sb[:, j * P:(j + 1) * P],
                    ident32[:BPC, :BPC],
                )
            hT_sb = cpool.tile([P, HB * BPC], f32)
            nc.vector.tensor_copy(hT_sb[:], hT_ps[:])

            # batch b's exp(logits) live at partition 32*b (legal base partitions).
            # softmax is shift-invariant and |logit| <~ 2 here, so exp is taken
            # directly (no max subtraction), fused into the PSUM->SBUF copy.
            e_sb = cpool.tile([P, T], f32)
            s_part = cpool.tile([P, NG], f32)     # per-group partial sums
            s_sb = cpool.tile([P, 1], f32)
            aT_sb = cpool.tile([P, BPC * TPB], bf16)   # col = b*TPB + i
            ctx_sb = cpool.tile([P, D], f32)

            # ---------- main pipeline over 16 groups, 2-deep skew ----------
            groups = [(b, g) for b in range(BPC) for g in range(NG)]
            fT_of = {}
            sc_of = {}

            def stage_T(idx):
                b, g = groups[idx]
                fgrp = ft[(b, g)]
                tiles = []
                for j in range(DB):
                    fT_ps = ps_fT.tile([P, GT], bf16, tag="fTps")
                    for i in range(NT):
                        nc.tensor.transpose(
                            fT_ps[:, i * P:(i + 1) * P],
                            fgrp[:, i, j * P:(j + 1) * P],
                            ident16[:, :],
                        )
                    t = fTpool.tile([P, GT], bf16, tag=f"fT{j}")
                    nc.vector.tensor_copy(t[:], fT_ps[:])
                    tiles.append(t)
                fT_of[idx] = tiles

            def stage_MM(idx):
                b, g = groups[idx]
                tiles = fT_of.pop(idx)
                scs = []
                for ub_i in range(UB):
                    wfT_ps = ps_wfT.tile([P, GT], f32, tag="wfT")
                    for j in range(DB):
                        nc.tensor.matmul(
                            wfT_ps[:],
                            wk_sb[j][:, ub_i * P:(ub_i + 1) * P],
                            tiles[j][:],
                            start=(j == 0),
                            stop=(j == DB - 1),
                        )
                    sc = scpool.tile([P, GT], bf16, tag=f"sc{ub_i}")
                    nc.scalar.activation(
                        sc[:],
                        wfT_ps[:],
                        AF.Tanh,
                        bias=bias_sb[:, ub_i * BPC + b: ub_i * BPC + b + 1],
                        scale=1.0,
                    )
                    scs.append(sc)
                sc_of[idx] = scs

            def stage_LG(idx):
                b, g = groups[idx]
                scs = sc_of.pop(idx)
                lg_ps = ps_mm1.tile([1, GT], f32, tag="mm1")
                for ub_i in range(UB):
                    nc.tensor.matmul(
                        lg_ps[:],
                        vkT[:, ub_i:ub_i + 1],
                        scs[ub_i][:],
                        start=(ub_i == 0),
                        stop=(ub_i == UB - 1),
                    )
                # fused exp straight from PSUM; accum_out = per-group sum
                nc.scalar.activation(
                    e_sb[32 * b:32 * b + 1, g * GT:(g + 1) * GT], lg_ps[:],
                    AF.Exp, bias=0.0, scale=1.0,
                    accum_out=s_part[32 * b:32 * b + 1, g:g + 1],
                )

            def stage_SMX(b):
                # per-batch: e (unnormalized softmax numerator) feeds the attn
                # transposes + context directly; host divides by s afterwards.
                r32 = slice(32 * b, 32 * b + 1)
                nc.vector.reduce_sum(s_sb[r32, :], s_part[r32, :], axis=mybir.AxisListType.X)
                nc.sync.dma_start(attn_out[b:b + 1, :], e_sb[r32, :])
                aT_ps = ps_pre.tile([P, TPB], f32, tag="pre")
                for i in range(TPB):
                    nc.tensor.transpose(
                        aT_ps[:, i:i + 1],
                        e_sb[r32, i * P:(i + 1) * P],
                        ident32[r32, 32 * b:32 * b + 1],
                        tile_position=(32 * b, 0),
                    )
                nc.vector.tensor_copy(aT_sb[:, b * TPB:(b + 1) * TPB], aT_ps[:])
                cx_ps = ps_mm1.tile([1, D], f32, tag="mm1")
                for i in range(TPB):
                    g, ii = divmod(i, NT)
                    nc.tensor.matmul(
                        cx_ps[:],
                        aT_sb[:, b * TPB + i: b * TPB + i + 1],
                        ft[(b, g)][:, ii, :],
                        start=(i == 0),
                        stop=(i == TPB - 1),
                    )
                nc.scalar.copy(ctx_sb[32 * b:32 * b + 1, :], cx_ps[:])

            n = len(groups)
            # prologue: first two transpose stages, then the uh/bias preamble
            # (PE order: hT, T(0), T(1), uh-MM, uhT, MM(0) -- no long stalls)
            stage_T(0)
            stage_T(1)
            _PROLOGUE = True

            uh_ps = ps_pre.tile([BPC, U], f32, tag="pre")
            for j in range(HB):
                nc.tensor.matmul(
                    uh_ps[:],
                    hT_sb[:, j * BPC:(j + 1) * BPC],
                    uk_sb[j][:],
                    start=(j == 0),
                    stop=(j == HB - 1),
                )
            uh_sb = cpool.tile([BPC, U], f32)
            nc.vector.tensor_copy(uh_sb[:], uh_ps[:])
            uhT_ps = ps_pre.tile([P, UB * BPC], f32, tag="pre")
            for jb in range(UB):
                nc.tensor.transpose(
                    uhT_ps[:, jb * BPC:(jb + 1) * BPC],
                    uh_sb[:, jb * P:(jb + 1) * P],
                    ident32[:BPC, :BPC],
                )
            bias_sb = cpool.tile([P, UB * BPC], f32)
            for jb in range(UB):
                nc.vector.tensor_scalar_add(
                    bias_sb[:, jb * BPC:(jb + 1) * BPC],
                    uhT_ps[:, jb * BPC:(jb + 1) * BPC],
                    wubT[:, jb:jb + 1],
                )

            stage_MM(0)
            for k in range(2, n + 2):
                if k < n:
                    stage_T(k)
                if 1 <= k - 1 < n:
                    stage_MM(k - 1)
                lg_idx = k - 2
                stage_LG(lg_idx)
                if lg_idx % NG == NG - 1:
                    stage_SMX(lg_idx // NG)

            nc.sync.dma_start(ctx_out[:, :], ctx_sb[0:P:32, :])
            nc.sync.dma_start(s_out[:, :], s_sb[0:P:32, :])

    nc.compile()
    return nc


def _get_nc():
    if "nc" not in _CACHE:
        _CACHE["nc"] = _build()
    return _CACHE["nc"]


def _shard(features, hidden, Wk, Wb, Uk, Ub, Vk):
    f32 = np.float32
    features = np.asarray(features, dtype=f32).reshape(B, T, D)
    hidden = np.asarray(hidden, dtype=f32)
    Wk = np.ascontiguousarray(np.asarray(Wk, dtype=f32))
    Wb = np.ascontiguousarray(np.asarray(Wb, dtype=f32))
    Uk = np.ascontiguousarray(np.asarray(Uk, dtype=f32))
    Ub = np.ascontiguousarray(np.asarray(Ub, dtype=f32))
    Vk = np.ascontiguousarray(np.asarray(Vk, dtype=f32).reshape(U, 1))
    in_maps = []
    for c in range(N_CORES):
        sl = slice(c * BPC, (c + 1) * BPC)
        in_maps.append({
            "features": np.ascontiguousarray(features[sl]).reshape(BPC * T, D),
            "hidden": np.ascontiguousarray(hidden[sl]),
            "Wk": Wk, "Wb": Wb, "Uk": Uk, "Ub": Ub, "Vk": Vk,
        })
    return in_maps


def _run(in_maps, trace=False, tmpdir=None):
    from concourse.bass_utils import run_bass_kernel_spmd
    nc = _get_nc()
    return run_bass_kernel_spmd(
        nc, in_maps, core_ids=list(range(N_CORES)), trace=trace, tmpdir=tmpdir
    )


def _post(e, s, ctx_e):
    """Host-side gather math: apply the softmax denominator."""
    attn = e / s
    ctx = ctx_e / s
    return ctx, attn


def kernel(features, hidden, Wk, Wb, Uk, Ub, Vk, Vb=None, **_ignored):
    in_maps = _shard(features, hidden, Wk, Wb, Uk, Ub, Vk)
    res = _run(in_maps)
    ctx_e = np.concatenate([r["ctx"] for r in res.results], axis=0)
    e = np.concatenate([r["attn"] for r in res.results], axis=0)
    s = np.concatenate([r["s"] for r in res.results], axis=0)
    ctx, attn = _post(e, s, ctx_e)
    return ctx.astype(np.float32), attn.reshape(B, T, 1).astype(np.float32)
